# revision 6
# baseline (speedup 1.0000x reference)
"""Trainium2 Bass kernel for per-frame complex 5-tap deep-filter FIR.

Problem: spec [8, 3000, 481, 2] f32 complex spectrogram, coef [8, 3000, 96, 10]
per-frame complex FIR coefficients (5 real taps then 5 imag taps) over the
first 96 frequency bins.  out[b,t,f] = sum_k spec[b,t-4+k,f] * coef[b,t,f,k]
(complex, causal zero-padded) for f < 96; bins 96..480 pass through.

Sharding: pure data parallel -- batch b -> NeuronCore b (8 batches, 8 cores).

v2 layout: the band is staged deinterleaved on host (per frame: 96 reals then
96 imags) and the coefficients tap-plane-major (per frame: cr_0..cr_4 then
ci_0..ci_4, 96 bins each), so every FIR operand is a contiguous step-1 slice.
On device the band + coefs are converted fp32->fp16 on the Scalar engine and
the FIR runs on the Vector engine as plain fp16 tensor_tensor ops, which hit
the 2x perf mode (fp32 tensor_tensor is capped at 1x on TRN2).  Accumulators
stay fp16 (|acc| <= ~60, fp16 rounding ~1e-3 rel -- far inside the 2e-2
gate).  The Scalar engine then interleaves acc_r/acc_i into the fp32 output
tile.  The 385 pass-through bins never touch SBUF: a DRAM->DRAM DMA copies
them, queued on the sync HWDGE ring behind the loads; band stores ride the
scalar ring so their semaphore gates can't block loads.

Time is tiled [4,10,10] frames/partition; all SBUF tags are double-buffered
so tile i+1's loads/converts overlap tile i's FIR.
"""

import numpy as np

B = 8
T = 3000
F = 481
ROW = 2 * F        # 962 floats per output frame (interleaved r,i)
NB = 96            # deep-filter band bins
BANDW = 2 * NB     # 192 floats per frame of staged band (xr plane + xi plane)
NO = 5             # FIR taps
NCOEF = 2 * NO * NB  # 960 floats of coef per frame
PTW = ROW - BANDW  # 770 pass-through floats per frame

TS_LIST = [4, 10, 10]  # frames per partition for each time tile
TP = 128 * sum(TS_LIST)  # padded time (3072)
PAD = 4                # leading zero rows in the staged band (causal halo)

_CACHE = {}


def _build_module(repeat: int = 1):
    import concourse.bass as bass
    import concourse.bacc as bacc
    import concourse.mybir as mybir
    from concourse.tile import TileContext

    f32 = mybir.dt.float32
    f16 = mybir.dt.float16
    mult = mybir.AluOpType.mult
    add = mybir.AluOpType.add
    sub = mybir.AluOpType.subtract
    AP = bass.AP

    nc = bacc.Bacc("TRN2", target_bir_lowering=False, debug=False, num_devices=B)
    # staged band: DRAM row r corresponds to frame r - PAD (leading zeros).
    specb_h = nc.dram_tensor("specb", [TP + PAD, BANDW], f32, kind="ExternalInput")
    specpt_h = nc.dram_tensor("specpt", [TP, PTW], f32, kind="ExternalInput")
    coef_h = nc.dram_tensor("coef", [TP, NCOEF], f32, kind="ExternalInput")
    out_h = nc.dram_tensor("out", [TP, ROW], f32, kind="ExternalOutput")
    specpt_ap = specpt_h.ap()
    out_ap = out_h.ap()

    if repeat == 0:
        # I/O-overhead baseline for timing: one trivial DMA, no compute.
        with TileContext(nc) as tc:
            with tc.tile_pool(name="pool", bufs=1) as pool:
                t0 = pool.tile([1, 2], f32)
                nc.sync.dma_start(out=t0[:, :], in_=specb_h.ap()[0:1, 0:2])
                nc.sync.dma_start(out=out_ap[0:1, 0:2], in_=t0[:, :])
        nc.compile()
        return nc

    def emit_body(nc, tc, pool):
        ntiles = len(TS_LIST)
        xs16 = [None] * ntiles
        cs16 = [None] * ntiles
        bases = []
        base = 0
        # Pass-through bins 96..480 (rows < T only; rows T..TP are padding no
        # one reads): DRAM->DRAM chunks interleaved between the tile loads on
        # the sync ring, so the pass-through finishes well before the final
        # band stores instead of starving them at the tail.
        NPT = 8
        PT_ROWS = T  # 3000
        pt_emitted = 0

        def emit_pt(n):
            nonlocal pt_emitted
            for _ in range(n):
                if pt_emitted >= NPT:
                    return
                r0 = pt_emitted * PT_ROWS // NPT
                r1 = (pt_emitted + 1) * PT_ROWS // NPT
                nc.sync.dma_start(
                    out=out_ap[r0:r1, BANDW:ROW],
                    in_=specpt_ap[r0:r1, :],
                )
                pt_emitted += 1

        # Phase 1: all loads (sync ring) + fp32->fp16 converts (scalar engine).
        for i, TS in enumerate(TS_LIST):
            bases.append(base)
            xf = pool.tile([128, (TS + 4) * BANDW], f32, name="xf", bufs=2)
            cf = pool.tile([128, TS * NCOEF], f32, name="cf", bufs=2)
            x16 = pool.tile([128, (TS + 4) * BANDW], f16, name="x16", bufs=2)
            c16 = pool.tile([128, TS * NCOEF], f16, name="c16", bufs=2)
            # per-partition contiguous block: frames [base+p*TS-4, base+p*TS+TS)
            # of the band (+PAD row shift), and [base+p*TS, ..+TS) of coef.
            nc.sync.dma_start(
                out=xf[:, :],
                in_=AP(specb_h, base * BANDW,
                       [[TS * BANDW, 128], [1, (TS + 4) * BANDW]]),
            )
            nc.sync.dma_start(
                out=cf[:, :],
                in_=AP(coef_h, base * NCOEF,
                       [[TS * NCOEF, 128], [1, TS * NCOEF]]),
            )
            nc.scalar.copy(out=x16[:, :], in_=xf[:, :])
            nc.scalar.copy(out=c16[:, :], in_=cf[:, :])
            xs16[i] = x16
            cs16[i] = c16
            base += 128 * TS
            emit_pt(1)

        # Phase 2: FIR on DVE (fp16 2x mode), interleave on scalar, store.
        for i, TS in enumerate(TS_LIST):
            base = bases[i]
            xv = xs16[i].rearrange("p (fr w) -> p fr w", w=BANDW)
            cv = cs16[i].rearrange("p (s pl w) -> p s pl w", pl=2 * NO, w=NB)
            acc_r = pool.tile([128, TS * NB], f16, name="acc_r", bufs=2)
            acc_i = pool.tile([128, TS * NB], f16, name="acc_i", bufs=2)
            tmp = pool.tile([128, TS * NB], f16, name="tmp", bufs=2)
            ob = pool.tile([128, TS * BANDW], f32, name="ob", bufs=2)
            ar = acc_r.rearrange("p (s w) -> p s w", w=NB)
            ai = acc_i.rearrange("p (s w) -> p s w", w=NB)
            tm = tmp.rearrange("p (s w) -> p s w", w=NB)

            for k in range(NO):
                ur = xv[:, k : k + TS, 0:NB]
                ui = xv[:, k : k + TS, NB:BANDW]
                cr = cv[:, :, k, :]
                ci = cv[:, :, NO + k, :]
                if k == 0:
                    nc.vector.tensor_tensor(out=ar, in0=ur, in1=cr, op=mult)
                    nc.vector.tensor_tensor(out=ai, in0=ur, in1=ci, op=mult)
                else:
                    nc.vector.tensor_tensor(out=tm, in0=ur, in1=cr, op=mult)
                    nc.vector.tensor_tensor(out=ar, in0=ar, in1=tm, op=add)
                    nc.vector.tensor_tensor(out=tm, in0=ur, in1=ci, op=mult)
                    nc.vector.tensor_tensor(out=ai, in0=ai, in1=tm, op=add)
                nc.vector.tensor_tensor(out=tm, in0=ui, in1=ci, op=mult)
                nc.vector.tensor_tensor(out=ar, in0=ar, in1=tm, op=sub)
                nc.vector.tensor_tensor(out=tm, in0=ui, in1=cr, op=mult)
                nc.vector.tensor_tensor(out=ai, in0=ai, in1=tm, op=add)

            # interleave (fp16 -> fp32, stride-2 dest) on the scalar engine
            obv = ob.rearrange("p (s w c) -> p s w c", w=NB, c=2)
            nc.scalar.copy(out=obv[:, :, :, 0], in_=ar)
            nc.scalar.copy(out=obv[:, :, :, 1], in_=ai)

            # store band (first 192 floats of each output row), scalar ring
            nc.scalar.dma_start(
                out=AP(out_h, base * ROW, [[TS * ROW, 128], [ROW, TS], [1, BANDW]]),
                in_=ob[:, :],
            )

        # remaining pass-through chunks after all loads
        emit_pt(NPT)

    with TileContext(nc) as tc:
        with tc.tile_pool(name="pool", bufs=1) as pool:
            for _ in range(repeat):
                emit_body(nc, tc, pool)

    nc.compile()
    return nc


def _get_module(repeat: int = 1):
    if repeat not in _CACHE:
        _CACHE[repeat] = _build_module(repeat)
    return _CACHE[repeat]


def _stage_inputs(spec: np.ndarray, coef: np.ndarray):
    """Host-side staging: lossless layout reorders + zero padding only."""
    spec = np.ascontiguousarray(spec, dtype=np.float32)
    coef = np.ascontiguousarray(coef, dtype=np.float32)
    # deinterleaved band planes: [B, T, 2, 96] -> per frame xr(96) xi(96)
    specb = np.zeros((B, TP + PAD, BANDW), np.float32)
    specb[:, PAD : PAD + T] = (
        spec[:, :, :NB, :].transpose(0, 1, 3, 2).reshape(B, T, BANDW)
    )
    # pass-through columns (bins 96..480 interleaved) = float cols 192:962
    specpt = np.zeros((B, TP, PTW), np.float32)
    specpt[:, :T] = spec.reshape(B, T, ROW)[:, :, BANDW:]
    # tap-plane-major coefs: [B, T, 10, 96] -> cr_0..cr_4, ci_0..ci_4
    coefp = np.zeros((B, TP, NCOEF), np.float32)
    coefp[:, :T] = coef.transpose(0, 1, 3, 2).reshape(B, T, NCOEF)
    return specb, specpt, coefp


def kernel(spec: np.ndarray, coef: np.ndarray) -> np.ndarray:
    from concourse import bass_utils

    assert spec.shape == (B, T, F, 2) and coef.shape == (B, T, NB, 2 * NO)
    specb, specpt, coefp = _stage_inputs(spec, coef)

    nc = _get_module()
    in_maps = [
        {"specb": specb[b], "specpt": specpt[b], "coef": coefp[b]}
        for b in range(B)
    ]
    res = bass_utils.run_bass_kernel_spmd(nc, in_maps, core_ids=list(range(B)))
    out = np.empty((B, T, F, 2), np.float32)
    for b in range(B):
        out[b] = res.results[b]["out"][:T].reshape(T, F, 2)
    return out


# revision 14
# speedup vs baseline: 1.0845x; 1.0845x over previous
"""Trainium2 Bass kernel for per-frame complex 5-tap deep-filter FIR.

Problem: spec [8, 3000, 481, 2] f32 complex spectrogram, coef [8, 3000, 96, 10]
per-frame complex FIR coefficients (5 real taps then 5 imag taps) over the
first 96 frequency bins.  out[b,t,f] = sum_k spec[b,t-4+k,f] * coef[b,t,f,k]
(complex, causal zero-padded) for f < 96; bins 96..480 pass through.

Sharding: pure data parallel -- batch b -> NeuronCore b (8 batches, 8 cores).

v2 layout: the band is staged deinterleaved on host (per frame: 96 reals then
96 imags) and the coefficients tap-plane-major (per frame: cr_0..cr_4 then
ci_0..ci_4, 96 bins each), so every FIR operand is a contiguous step-1 slice.
On device the band + coefs are converted fp32->fp16 on the Scalar engine and
the FIR runs on the Vector engine as plain fp16 tensor_tensor ops, which hit
the 2x perf mode (fp32 tensor_tensor is capped at 1x on TRN2).  Accumulators
stay fp16 (|acc| <= ~60, fp16 rounding ~1e-3 rel -- far inside the 2e-2
gate).  The Scalar engine then interleaves acc_r/acc_i into the fp32 output
tile.  The 385 pass-through bins never touch SBUF: a DRAM->DRAM DMA copies
them, queued on the sync HWDGE ring behind the loads; band stores ride the
scalar ring so their semaphore gates can't block loads.

Time is tiled [4,10,10] frames/partition; all SBUF tags are double-buffered
so tile i+1's loads/converts overlap tile i's FIR.
"""

import numpy as np

B = 8
T = 3000
F = 481
ROW = 2 * F        # 962 floats per output frame (interleaved r,i)
NB = 96            # deep-filter band bins
BANDW = 2 * NB     # 192 floats per frame of staged band (xr plane + xi plane)
NO = 5             # FIR taps
NCOEF = 2 * NO * NB  # 960 floats of coef per frame
PTW = ROW - BANDW  # 770 pass-through floats per frame

TS_LIST = [4, 8, 8, 4]  # frames per partition for each time tile
# After which tile index to emit one pass-through chunk (rest go after all
# loads).  Tuned on HW: dense loads + all pass-through at the end wins
# (small first tile starts the FIR early, small last tile shortens the tail).
PT_AFTER = ()
TP = 128 * 24          # padded time (3072); every TS_LIST must sum to 24
PAD = 4                # leading zero rows in the staged band (causal halo)

_CACHE = {}


def _build_module(repeat: int = 1, ts_list=None, pt_after=None):
    import concourse.bass as bass
    import concourse.bacc as bacc
    import concourse.mybir as mybir
    from concourse.tile import TileContext

    ts_list = list(TS_LIST if ts_list is None else ts_list)
    pt_after = tuple(PT_AFTER if pt_after is None else pt_after)
    assert sum(ts_list) * 128 == TP

    f32 = mybir.dt.float32
    f16 = mybir.dt.float16
    mult = mybir.AluOpType.mult
    add = mybir.AluOpType.add
    sub = mybir.AluOpType.subtract
    AP = bass.AP

    nc = bacc.Bacc("TRN2", target_bir_lowering=False, debug=False, num_devices=B)
    # staged band: DRAM row r corresponds to frame r - PAD (leading zeros).
    specb_h = nc.dram_tensor("specb", [TP + PAD, BANDW], f32, kind="ExternalInput")
    specpt_h = nc.dram_tensor("specpt", [TP, PTW], f32, kind="ExternalInput")
    coef_h = nc.dram_tensor("coef", [TP, NCOEF], f32, kind="ExternalInput")
    out_h = nc.dram_tensor("out", [TP, ROW], f32, kind="ExternalOutput")
    specpt_ap = specpt_h.ap()
    out_ap = out_h.ap()

    if repeat == 0:
        # I/O-overhead baseline for timing: one trivial DMA, no compute.
        with TileContext(nc) as tc:
            with tc.tile_pool(name="pool", bufs=1) as pool:
                t0 = pool.tile([1, 2], f32)
                nc.sync.dma_start(out=t0[:, :], in_=specb_h.ap()[0:1, 0:2])
                nc.sync.dma_start(out=out_ap[0:1, 0:2], in_=t0[:, :])
        nc.compile()
        return nc

    def emit_body(nc, tc, pool):
        ntiles = len(ts_list)
        xs16 = [None] * ntiles
        cs16 = [None] * ntiles
        bases = []
        base = 0
        # Pass-through bins 96..480 (rows < T only; rows T..TP are padding no
        # one reads): DRAM->DRAM chunks interleaved between the tile loads on
        # the sync ring, so the pass-through finishes well before the final
        # band stores instead of starving them at the tail.
        NPT = 8
        PT_ROWS = T  # 3000
        pt_emitted = 0

        def emit_pt(n):
            nonlocal pt_emitted
            for _ in range(n):
                if pt_emitted >= NPT:
                    return
                r0 = pt_emitted * PT_ROWS // NPT
                r1 = (pt_emitted + 1) * PT_ROWS // NPT
                nc.sync.dma_start(
                    out=out_ap[r0:r1, BANDW:ROW],
                    in_=specpt_ap[r0:r1, :],
                )
                pt_emitted += 1

        # Phase 1: all loads (sync ring) + fp32->fp16 converts (scalar engine).
        for i, TS in enumerate(ts_list):
            bases.append(base)
            xf = pool.tile([128, (TS + 4) * BANDW], f32, name="xf", bufs=2)
            cf = pool.tile([128, TS * NCOEF], f32, name="cf", bufs=2)
            x16 = pool.tile([128, (TS + 4) * BANDW], f16, name="x16", bufs=2)
            c16 = pool.tile([128, TS * NCOEF], f16, name="c16", bufs=2)
            # per-partition contiguous block: frames [base+p*TS-4, base+p*TS+TS)
            # of the band (+PAD row shift), and [base+p*TS, ..+TS) of coef.
            nc.sync.dma_start(
                out=xf[:, :],
                in_=AP(specb_h, base * BANDW,
                       [[TS * BANDW, 128], [1, (TS + 4) * BANDW]]),
            )
            nc.sync.dma_start(
                out=cf[:, :],
                in_=AP(coef_h, base * NCOEF,
                       [[TS * NCOEF, 128], [1, TS * NCOEF]]),
            )
            nc.scalar.copy(out=x16[:, :], in_=xf[:, :])
            nc.scalar.copy(out=c16[:, :], in_=cf[:, :])
            xs16[i] = x16
            cs16[i] = c16
            base += 128 * TS
            if i in pt_after:
                emit_pt(1)

        # Phase 2: FIR on DVE (fp16 2x mode), interleave on scalar, store.
        for i, TS in enumerate(ts_list):
            base = bases[i]
            xv = xs16[i].rearrange("p (fr w) -> p fr w", w=BANDW)
            cv = cs16[i].rearrange("p (s pl w) -> p s pl w", pl=2 * NO, w=NB)
            acc_r = pool.tile([128, TS * NB], f16, name="acc_r", bufs=2)
            acc_i = pool.tile([128, TS * NB], f16, name="acc_i", bufs=2)
            tmp = pool.tile([128, TS * NB], f16, name="tmp", bufs=2)
            ob = pool.tile([128, TS * BANDW], f32, name="ob", bufs=2)
            ar = acc_r.rearrange("p (s w) -> p s w", w=NB)
            ai = acc_i.rearrange("p (s w) -> p s w", w=NB)
            tm = tmp.rearrange("p (s w) -> p s w", w=NB)

            for k in range(NO):
                ur = xv[:, k : k + TS, 0:NB]
                ui = xv[:, k : k + TS, NB:BANDW]
                cr = cv[:, :, k, :]
                ci = cv[:, :, NO + k, :]
                if k == 0:
                    nc.vector.tensor_tensor(out=ar, in0=ur, in1=cr, op=mult)
                    nc.vector.tensor_tensor(out=ai, in0=ur, in1=ci, op=mult)
                else:
                    nc.vector.tensor_tensor(out=tm, in0=ur, in1=cr, op=mult)
                    nc.vector.tensor_tensor(out=ar, in0=ar, in1=tm, op=add)
                    nc.vector.tensor_tensor(out=tm, in0=ur, in1=ci, op=mult)
                    nc.vector.tensor_tensor(out=ai, in0=ai, in1=tm, op=add)
                nc.vector.tensor_tensor(out=tm, in0=ui, in1=ci, op=mult)
                nc.vector.tensor_tensor(out=ar, in0=ar, in1=tm, op=sub)
                nc.vector.tensor_tensor(out=tm, in0=ui, in1=cr, op=mult)
                nc.vector.tensor_tensor(out=ai, in0=ai, in1=tm, op=add)

            # interleave (fp16 -> fp32, stride-2 dest) on the scalar engine
            obv = ob.rearrange("p (s w c) -> p s w c", w=NB, c=2)
            nc.scalar.copy(out=obv[:, :, :, 0], in_=ar)
            nc.scalar.copy(out=obv[:, :, :, 1], in_=ai)

            # store band (first 192 floats of each output row), scalar ring
            nc.scalar.dma_start(
                out=AP(out_h, base * ROW, [[TS * ROW, 128], [ROW, TS], [1, BANDW]]),
                in_=ob[:, :],
            )

        # remaining pass-through chunks after all loads
        emit_pt(NPT)

    with TileContext(nc) as tc:
        with tc.tile_pool(name="pool", bufs=1) as pool:
            for _ in range(repeat):
                emit_body(nc, tc, pool)

    nc.compile()
    return nc


def _get_module(repeat: int = 1, ts_list=None, pt_after=None):
    key = (repeat, tuple(ts_list) if ts_list else None,
           tuple(pt_after) if pt_after is not None else None)
    if key not in _CACHE:
        _CACHE[key] = _build_module(repeat, ts_list, pt_after)
    return _CACHE[key]


def _stage_inputs(spec: np.ndarray, coef: np.ndarray):
    """Host-side staging: lossless layout reorders + zero padding only."""
    spec = np.ascontiguousarray(spec, dtype=np.float32)
    coef = np.ascontiguousarray(coef, dtype=np.float32)
    # deinterleaved band planes: [B, T, 2, 96] -> per frame xr(96) xi(96)
    specb = np.zeros((B, TP + PAD, BANDW), np.float32)
    specb[:, PAD : PAD + T] = (
        spec[:, :, :NB, :].transpose(0, 1, 3, 2).reshape(B, T, BANDW)
    )
    # pass-through columns (bins 96..480 interleaved) = float cols 192:962
    specpt = np.zeros((B, TP, PTW), np.float32)
    specpt[:, :T] = spec.reshape(B, T, ROW)[:, :, BANDW:]
    # tap-plane-major coefs: [B, T, 10, 96] -> cr_0..cr_4, ci_0..ci_4
    coefp = np.zeros((B, TP, NCOEF), np.float32)
    coefp[:, :T] = coef.transpose(0, 1, 3, 2).reshape(B, T, NCOEF)
    return specb, specpt, coefp


def kernel(spec: np.ndarray, coef: np.ndarray) -> np.ndarray:
    from concourse import bass_utils

    assert spec.shape == (B, T, F, 2) and coef.shape == (B, T, NB, 2 * NO)
    specb, specpt, coefp = _stage_inputs(spec, coef)

    nc = _get_module()
    in_maps = [
        {"specb": specb[b], "specpt": specpt[b], "coef": coefp[b]}
        for b in range(B)
    ]
    res = bass_utils.run_bass_kernel_spmd(nc, in_maps, core_ids=list(range(B)))
    out = np.empty((B, T, F, 2), np.float32)
    for b in range(B):
        out[b] = res.results[b]["out"][:T].reshape(T, F, 2)
    return out


# revision 22
# speedup vs baseline: 1.1112x; 1.0246x over previous
"""Trainium2 Bass kernel for per-frame complex 5-tap deep-filter FIR.

Problem: spec [8, 3000, 481, 2] f32 complex spectrogram, coef [8, 3000, 96, 10]
per-frame complex FIR coefficients (5 real taps then 5 imag taps) over the
first 96 frequency bins.  out[b,t,f] = sum_k spec[b,t-4+k,f] * coef[b,t,f,k]
(complex, causal zero-padded) for f < 96; bins 96..480 pass through.

Sharding: pure data parallel -- batch b -> NeuronCore b (8 batches, 8 cores).

v2 layout: the band is staged deinterleaved on host (per frame: 96 reals then
96 imags) and the coefficients tap-plane-major (per frame: cr_0..cr_4 then
ci_0..ci_4, 96 bins each), so every FIR operand is a contiguous step-1 slice.
On device the band + coefs are converted fp32->fp16 on the Scalar engine and
the FIR runs on the Vector engine as plain fp16 tensor_tensor ops, which hit
the 2x perf mode (fp32 tensor_tensor is capped at 1x on TRN2).  Accumulators
stay fp16 (|acc| <= ~60, fp16 rounding ~1e-3 rel -- far inside the 2e-2
gate).  The Scalar engine then interleaves acc_r/acc_i into the fp32 output
tile.  The 385 pass-through bins never touch SBUF: a DRAM->DRAM DMA copies
them, queued on the sync HWDGE ring behind the loads; band stores ride the
scalar ring so their semaphore gates can't block loads.

Time is tiled [4,8,8,4] frames/partition (small first tile starts the FIR
early, small last tile shortens the interleave/store tail); all SBUF tags are
double-buffered so tile i+1's loads/converts overlap tile i's FIR.
"""

import numpy as np

B = 8
T = 3000
F = 481
ROW = 2 * F        # 962 floats per output frame (interleaved r,i)
NB = 96            # deep-filter band bins
BANDW = 2 * NB     # 192 floats per frame of staged band (xr plane + xi plane)
NO = 5             # FIR taps
NCOEF = 2 * NO * NB  # 960 floats of coef per frame
PTW = ROW - BANDW  # 770 pass-through floats per frame

TS_LIST = [4, 8, 8, 4]  # frames per partition for each time tile
# After which tile index to emit one pass-through chunk (rest go after all
# loads).  Tuned on HW: dense loads + all pass-through at the end wins
# (small first tile starts the FIR early, small last tile shortens the tail).
PT_AFTER = ()
TP = 128 * 24          # padded time (3072); every TS_LIST must sum to 24
PAD = 4                # leading zero rows in the staged band (causal halo)

_CACHE = {}


GP_TAPS = ()  # taps whose partial accumulation runs on GpSimd (tuned on HW)
NPT = 8       # pass-through DRAM->DRAM chunk count


def _build_module(repeat: int = 1, ts_list=None, pt_after=None, gp_taps=None,
                  npt=None):
    import concourse.bass as bass
    import concourse.bacc as bacc
    import concourse.mybir as mybir
    from concourse.tile import TileContext

    ts_list = list(TS_LIST if ts_list is None else ts_list)
    pt_after = tuple(PT_AFTER if pt_after is None else pt_after)
    gp_taps = tuple(GP_TAPS if gp_taps is None else gp_taps)
    npt = NPT if npt is None else npt
    assert sum(ts_list) * 128 == TP

    f32 = mybir.dt.float32
    f16 = mybir.dt.float16
    mult = mybir.AluOpType.mult
    add = mybir.AluOpType.add
    sub = mybir.AluOpType.subtract
    AP = bass.AP

    nc = bacc.Bacc("TRN2", target_bir_lowering=False, debug=False, num_devices=B)
    # staged band: DRAM row r corresponds to frame r - PAD (leading zeros).
    specb_h = nc.dram_tensor("specb", [TP + PAD, BANDW], f32, kind="ExternalInput")
    specpt_h = nc.dram_tensor("specpt", [TP, PTW], f32, kind="ExternalInput")
    coef_h = nc.dram_tensor("coef", [TP, NCOEF], f32, kind="ExternalInput")
    out_h = nc.dram_tensor("out", [TP, ROW], f32, kind="ExternalOutput")
    specpt_ap = specpt_h.ap()
    out_ap = out_h.ap()

    if repeat == 0:
        # I/O-overhead baseline for timing: one trivial DMA, no compute.
        with TileContext(nc) as tc:
            with tc.tile_pool(name="pool", bufs=1) as pool:
                t0 = pool.tile([1, 2], f32)
                nc.sync.dma_start(out=t0[:, :], in_=specb_h.ap()[0:1, 0:2])
                nc.sync.dma_start(out=out_ap[0:1, 0:2], in_=t0[:, :])
        nc.compile()
        return nc

    def emit_body(nc, tc, pool):
        ntiles = len(ts_list)
        xs16 = [None] * ntiles
        cs16 = [None] * ntiles
        bases = []
        base = 0
        # Pass-through bins 96..480 (rows < T only; rows T..TP are padding no
        # one reads): DRAM->DRAM chunks on the sync ring, behind the loads
        # (pt_after can interleave some chunks between tile loads; measured
        # fastest with all chunks after the last load).
        PT_ROWS = T  # 3000
        pt_emitted = 0

        def emit_pt(n):
            nonlocal pt_emitted
            for _ in range(n):
                if pt_emitted >= npt:
                    return
                r0 = pt_emitted * PT_ROWS // npt
                r1 = (pt_emitted + 1) * PT_ROWS // npt
                nc.sync.dma_start(
                    out=out_ap[r0:r1, BANDW:ROW],
                    in_=specpt_ap[r0:r1, :],
                )
                pt_emitted += 1

        # Phase 1: all loads (sync ring) + fp32->fp16 converts (scalar engine).
        for i, TS in enumerate(ts_list):
            bases.append(base)
            xf = pool.tile([128, (TS + 4) * BANDW], f32, name="xf", bufs=2)
            cf = pool.tile([128, TS * NCOEF], f32, name="cf", bufs=2)
            x16 = pool.tile([128, (TS + 4) * BANDW], f16, name="x16", bufs=2)
            c16 = pool.tile([128, TS * NCOEF], f16, name="c16", bufs=2)
            # per-partition contiguous block: frames [base+p*TS-4, base+p*TS+TS)
            # of the band (+PAD row shift), and [base+p*TS, ..+TS) of coef.
            nc.sync.dma_start(
                out=xf[:, :],
                in_=AP(specb_h, base * BANDW,
                       [[TS * BANDW, 128], [1, (TS + 4) * BANDW]]),
            )
            nc.sync.dma_start(
                out=cf[:, :],
                in_=AP(coef_h, base * NCOEF,
                       [[TS * NCOEF, 128], [1, TS * NCOEF]]),
            )
            nc.scalar.copy(out=x16[:, :], in_=xf[:, :])
            nc.scalar.copy(out=c16[:, :], in_=cf[:, :])
            xs16[i] = x16
            cs16[i] = c16
            base += 128 * TS
            if i in pt_after:
                emit_pt(1)

        # Phase 2: FIR on DVE (fp16 2x mode), interleave on scalar, store.
        for i, TS in enumerate(ts_list):
            base = bases[i]
            xv = xs16[i].rearrange("p (fr w) -> p fr w", w=BANDW)
            cv = cs16[i].rearrange("p (s pl w) -> p s pl w", pl=2 * NO, w=NB)
            acc_r = pool.tile([128, TS * NB], f16, name="acc_r", bufs=2)
            acc_i = pool.tile([128, TS * NB], f16, name="acc_i", bufs=2)
            tmp = pool.tile([128, TS * NB], f16, name="tmp", bufs=2)
            ob = pool.tile([128, TS * BANDW], f32, name="ob", bufs=2)
            ar = acc_r.rearrange("p (s w) -> p s w", w=NB)
            ai = acc_i.rearrange("p (s w) -> p s w", w=NB)
            tm = tmp.rearrange("p (s w) -> p s w", w=NB)

            if gp_taps:
                gacc_r = pool.tile([128, TS * NB], f16, name="gacc_r", bufs=2)
                gacc_i = pool.tile([128, TS * NB], f16, name="gacc_i", bufs=2)
                gtmp = pool.tile([128, TS * NB], f16, name="gtmp", bufs=2)
                gar = gacc_r.rearrange("p (s w) -> p s w", w=NB)
                gai = gacc_i.rearrange("p (s w) -> p s w", w=NB)
                gtm = gtmp.rearrange("p (s w) -> p s w", w=NB)

            first_v = True
            first_g = True
            for k in range(NO):
                ur = xv[:, k : k + TS, 0:NB]
                ui = xv[:, k : k + TS, NB:BANDW]
                cr = cv[:, :, k, :]
                ci = cv[:, :, NO + k, :]
                if k in gp_taps:
                    eng, a_r, a_i, t_m, first = (
                        nc.gpsimd, gar, gai, gtm, first_g)
                    first_g = False
                else:
                    eng, a_r, a_i, t_m, first = (
                        nc.vector, ar, ai, tm, first_v)
                    first_v = False
                if first:
                    eng.tensor_tensor(out=a_r, in0=ur, in1=cr, op=mult)
                    eng.tensor_tensor(out=a_i, in0=ur, in1=ci, op=mult)
                else:
                    eng.tensor_tensor(out=t_m, in0=ur, in1=cr, op=mult)
                    eng.tensor_tensor(out=a_r, in0=a_r, in1=t_m, op=add)
                    eng.tensor_tensor(out=t_m, in0=ur, in1=ci, op=mult)
                    eng.tensor_tensor(out=a_i, in0=a_i, in1=t_m, op=add)
                eng.tensor_tensor(out=t_m, in0=ui, in1=ci, op=mult)
                eng.tensor_tensor(out=a_r, in0=a_r, in1=t_m, op=sub)
                eng.tensor_tensor(out=t_m, in0=ui, in1=cr, op=mult)
                eng.tensor_tensor(out=a_i, in0=a_i, in1=t_m, op=add)
            if gp_taps:
                # merge the GpSimd partial accumulators (on DVE)
                nc.vector.tensor_tensor(out=ar, in0=ar, in1=gar, op=add)
                nc.vector.tensor_tensor(out=ai, in0=ai, in1=gai, op=add)

            # interleave (fp16 -> fp32, stride-2 dest) on the scalar engine
            obv = ob.rearrange("p (s w c) -> p s w c", w=NB, c=2)
            nc.scalar.copy(out=obv[:, :, :, 0], in_=ar)
            nc.scalar.copy(out=obv[:, :, :, 1], in_=ai)

            # store band (first 192 floats of each output row), scalar ring
            nc.scalar.dma_start(
                out=AP(out_h, base * ROW, [[TS * ROW, 128], [ROW, TS], [1, BANDW]]),
                in_=ob[:, :],
            )

        # remaining pass-through chunks after all loads
        emit_pt(npt)

    with TileContext(nc) as tc:
        with tc.tile_pool(name="pool", bufs=1) as pool:
            for _ in range(repeat):
                emit_body(nc, tc, pool)

    nc.compile()
    return nc


def _get_module(repeat: int = 1, ts_list=None, pt_after=None, gp_taps=None,
                npt=None):
    key = (repeat, tuple(ts_list) if ts_list else None,
           tuple(pt_after) if pt_after is not None else None,
           tuple(gp_taps) if gp_taps is not None else None, npt)
    if key not in _CACHE:
        _CACHE[key] = _build_module(repeat, ts_list, pt_after, gp_taps, npt)
    return _CACHE[key]


def _stage_inputs(spec: np.ndarray, coef: np.ndarray):
    """Host-side staging: lossless layout reorders + zero padding only."""
    spec = np.ascontiguousarray(spec, dtype=np.float32)
    coef = np.ascontiguousarray(coef, dtype=np.float32)
    # deinterleaved band planes: [B, T, 2, 96] -> per frame xr(96) xi(96)
    specb = np.zeros((B, TP + PAD, BANDW), np.float32)
    specb[:, PAD : PAD + T] = (
        spec[:, :, :NB, :].transpose(0, 1, 3, 2).reshape(B, T, BANDW)
    )
    # pass-through columns (bins 96..480 interleaved) = float cols 192:962
    specpt = np.zeros((B, TP, PTW), np.float32)
    specpt[:, :T] = spec.reshape(B, T, ROW)[:, :, BANDW:]
    # tap-plane-major coefs: [B, T, 10, 96] -> cr_0..cr_4, ci_0..ci_4
    coefp = np.zeros((B, TP, NCOEF), np.float32)
    coefp[:, :T] = coef.transpose(0, 1, 3, 2).reshape(B, T, NCOEF)
    return specb, specpt, coefp


def kernel(spec: np.ndarray, coef: np.ndarray) -> np.ndarray:
    from concourse import bass_utils

    assert spec.shape == (B, T, F, 2) and coef.shape == (B, T, NB, 2 * NO)
    specb, specpt, coefp = _stage_inputs(spec, coef)

    nc = _get_module()
    in_maps = [
        {"specb": specb[b], "specpt": specpt[b], "coef": coefp[b]}
        for b in range(B)
    ]
    res = bass_utils.run_bass_kernel_spmd(nc, in_maps, core_ids=list(range(B)))
    out = np.empty((B, T, F, 2), np.float32)
    for b in range(B):
        out[b] = res.results[b]["out"][:T].reshape(T, F, 2)
    return out


# revision 30
# speedup vs baseline: 1.1284x; 1.0155x over previous
"""Trainium2 Bass kernel for per-frame complex 5-tap deep-filter FIR.

Problem: spec [8, 3000, 481, 2] f32 complex spectrogram, coef [8, 3000, 96, 10]
per-frame complex FIR coefficients (5 real taps then 5 imag taps) over the
first 96 frequency bins.  out[b,t,f] = sum_k spec[b,t-4+k,f] * coef[b,t,f,k]
(complex, causal zero-padded) for f < 96; bins 96..480 pass through.

Sharding: pure data parallel -- batch b -> NeuronCore b (8 batches, 8 cores).

v2 layout: the band is staged deinterleaved on host (per frame: 96 reals then
96 imags) and the coefficients tap-plane-major (per frame: cr_0..cr_4 then
ci_0..ci_4, 96 bins each), so every FIR operand is a contiguous step-1 slice.
On device the band + coefs are converted fp32->fp16 on the Scalar engine and
the FIR runs on the Vector engine as plain fp16 tensor_tensor ops, which hit
the 2x perf mode (fp32 tensor_tensor is capped at 1x on TRN2).  Accumulators
stay fp16 (|acc| <= ~60, fp16 rounding ~1e-3 rel -- far inside the 2e-2
gate).  The Scalar engine then interleaves acc_r/acc_i into the fp32 output
tile.  The 385 pass-through bins never touch SBUF: a DRAM->DRAM DMA copies
them, queued on the sync HWDGE ring behind the loads; band stores ride the
scalar ring so their semaphore gates can't block loads.

Time is tiled [4,8,8,4] frames/partition (small first tile starts the FIR
early, small last tile shortens the interleave/store tail); all SBUF tags are
double-buffered so tile i+1's loads/converts overlap tile i's FIR.
"""

import numpy as np

B = 8
T = 3000
F = 481
ROW = 2 * F        # 962 floats per output frame (interleaved r,i)
NB = 96            # deep-filter band bins
BANDW = 2 * NB     # 192 floats per frame of staged band (xr plane + xi plane)
NO = 5             # FIR taps
NCOEF = 2 * NO * NB  # 960 floats of coef per frame
PTW = ROW - BANDW  # 770 pass-through floats per frame

TS_LIST = [2, 8, 8, 6]  # frames per partition for each time tile
# After which tile index to emit one pass-through chunk (rest go after all
# loads).  Tuned on HW: dense loads + all pass-through at the end wins
# (small first tile starts the FIR early; with the paired FIR the DVE chain
# is short enough that a tiny first tile beats balancing the last one).
PT_AFTER = ()
TP = 128 * 24          # padded time (3072); every TS_LIST must sum to 24
PAD = 4                # leading zero rows in the staged band (causal halo)

_CACHE = {}


GP_TAPS = ()  # taps whose partial accumulation runs on GpSimd (tuned on HW)
NPT = 8       # pass-through DRAM->DRAM chunk count
PAIRED = True  # pair-fused products: 2 double-width TT ops per tap instead of 4


def _build_module(repeat: int = 1, ts_list=None, pt_after=None, gp_taps=None,
                  npt=None, paired=None):
    import concourse.bass as bass
    import concourse.bacc as bacc
    import concourse.mybir as mybir
    from concourse.tile import TileContext

    ts_list = list(TS_LIST if ts_list is None else ts_list)
    pt_after = tuple(PT_AFTER if pt_after is None else pt_after)
    gp_taps = tuple(GP_TAPS if gp_taps is None else gp_taps)
    npt = NPT if npt is None else npt
    paired = PAIRED if paired is None else paired
    assert sum(ts_list) * 128 == TP

    f32 = mybir.dt.float32
    f16 = mybir.dt.float16
    mult = mybir.AluOpType.mult
    add = mybir.AluOpType.add
    sub = mybir.AluOpType.subtract
    AP = bass.AP

    nc = bacc.Bacc("TRN2", target_bir_lowering=False, debug=False, num_devices=B)
    # staged band: DRAM row r corresponds to frame r - PAD (leading zeros).
    specb_h = nc.dram_tensor("specb", [TP + PAD, BANDW], f32, kind="ExternalInput")
    specpt_h = nc.dram_tensor("specpt", [TP, PTW], f32, kind="ExternalInput")
    coef_h = nc.dram_tensor("coef", [TP, NCOEF], f32, kind="ExternalInput")
    out_h = nc.dram_tensor("out", [TP, ROW], f32, kind="ExternalOutput")
    specpt_ap = specpt_h.ap()
    out_ap = out_h.ap()

    if repeat == 0:
        # I/O-overhead baseline for timing: one trivial DMA, no compute.
        with TileContext(nc) as tc:
            with tc.tile_pool(name="pool", bufs=1) as pool:
                t0 = pool.tile([1, 2], f32)
                nc.sync.dma_start(out=t0[:, :], in_=specb_h.ap()[0:1, 0:2])
                nc.sync.dma_start(out=out_ap[0:1, 0:2], in_=t0[:, :])
        nc.compile()
        return nc

    def emit_body(nc, tc, pool):
        ntiles = len(ts_list)
        xs16 = [None] * ntiles
        cs16 = [None] * ntiles
        bases = []
        base = 0
        # Pass-through bins 96..480 (rows < T only; rows T..TP are padding no
        # one reads): DRAM->DRAM chunks on the sync ring, behind the loads
        # (pt_after can interleave some chunks between tile loads; measured
        # fastest with all chunks after the last load).
        PT_ROWS = T  # 3000
        pt_emitted = 0

        def emit_pt(n):
            nonlocal pt_emitted
            for _ in range(n):
                if pt_emitted >= npt:
                    return
                r0 = pt_emitted * PT_ROWS // npt
                r1 = (pt_emitted + 1) * PT_ROWS // npt
                nc.sync.dma_start(
                    out=out_ap[r0:r1, BANDW:ROW],
                    in_=specpt_ap[r0:r1, :],
                )
                pt_emitted += 1

        # Phase 1: all loads (sync ring) + fp32->fp16 converts (scalar engine).
        for i, TS in enumerate(ts_list):
            bases.append(base)
            xf = pool.tile([128, (TS + 4) * BANDW], f32, name="xf", bufs=2)
            cf = pool.tile([128, TS * NCOEF], f32, name="cf", bufs=2)
            x16 = pool.tile([128, (TS + 4) * BANDW], f16, name="x16", bufs=2)
            c16 = pool.tile([128, TS * NCOEF], f16, name="c16", bufs=2)
            # per-partition contiguous block: frames [base+p*TS-4, base+p*TS+TS)
            # of the band (+PAD row shift), and [base+p*TS, ..+TS) of coef.
            nc.sync.dma_start(
                out=xf[:, :],
                in_=AP(specb_h, base * BANDW,
                       [[TS * BANDW, 128], [1, (TS + 4) * BANDW]]),
            )
            nc.sync.dma_start(
                out=cf[:, :],
                in_=AP(coef_h, base * NCOEF,
                       [[TS * NCOEF, 128], [1, TS * NCOEF]]),
            )
            nc.scalar.copy(out=x16[:, :], in_=xf[:, :])
            nc.scalar.copy(out=c16[:, :], in_=cf[:, :])
            if paired:
                # half-swapped band copy [xi|xr] per frame, built straight
                # from the fp32 tile (converts + swaps in two ACT copies)
                x2 = pool.tile([128, (TS + 4) * BANDW], f16, name="x2", bufs=2)
                xfv = xf.rearrange("p (fr w) -> p fr w", w=BANDW)
                x2v = x2.rearrange("p (fr w) -> p fr w", w=BANDW)
                nc.scalar.copy(out=x2v[:, :, NB:BANDW], in_=xfv[:, :, 0:NB])
                nc.scalar.copy(out=x2v[:, :, 0:NB], in_=xfv[:, :, NB:BANDW])
                xs16[i] = (x16, x2)
            else:
                xs16[i] = (x16, None)
            cs16[i] = c16
            base += 128 * TS
            if i in pt_after:
                emit_pt(1)

        # Phase 2: FIR on DVE (fp16 2x mode), interleave on scalar, store.
        for i, TS in enumerate(ts_list):
            base = bases[i]
            xv = xs16[i][0].rearrange("p (fr w) -> p fr w", w=BANDW)
            acc_r = pool.tile([128, TS * NB], f16, name="acc_r", bufs=2)
            acc_i = pool.tile([128, TS * NB], f16, name="acc_i", bufs=2)
            ob = pool.tile([128, TS * BANDW], f32, name="ob", bufs=2)
            ar = acc_r.rearrange("p (s w) -> p s w", w=NB)
            ai = acc_i.rearrange("p (s w) -> p s w", w=NB)

            if paired:
                # coef planes are pair-major: plane k = [cr_k(96) | ci_k(96)].
                # One 192-wide op per (tap, accumulator):
                #   a2r += [xr|xi]*[cr|ci] = [ur*cr | ui*ci]
                #   a2i += [xi|xr]*[cr|ci] = [ui*cr | ur*ci]
                # then fold halves: acc_r = lo-hi, acc_i = lo+hi.
                x2v = xs16[i][1].rearrange("p (fr w) -> p fr w", w=BANDW)
                cpv = cs16[i].rearrange("p (s pl w) -> p s pl w", pl=NO, w=BANDW)
                a2r_t = pool.tile([128, TS * BANDW], f16, name="a2r", bufs=2)
                a2i_t = pool.tile([128, TS * BANDW], f16, name="a2i", bufs=2)
                t2_t = pool.tile([128, TS * BANDW], f16, name="t2", bufs=2)
                a2r = a2r_t.rearrange("p (s w) -> p s w", w=BANDW)
                a2i = a2i_t.rearrange("p (s w) -> p s w", w=BANDW)
                t2 = t2_t.rearrange("p (s w) -> p s w", w=BANDW)
                for k in range(NO):
                    ua = xv[:, k : k + TS, :]
                    ub = x2v[:, k : k + TS, :]
                    cp = cpv[:, :, k, :]
                    if k == 0:
                        nc.vector.tensor_tensor(out=a2r, in0=ua, in1=cp, op=mult)
                        nc.vector.tensor_tensor(out=a2i, in0=ub, in1=cp, op=mult)
                    else:
                        nc.vector.tensor_tensor(out=t2, in0=ua, in1=cp, op=mult)
                        nc.vector.tensor_tensor(out=a2r, in0=a2r, in1=t2, op=add)
                        nc.vector.tensor_tensor(out=t2, in0=ub, in1=cp, op=mult)
                        nc.vector.tensor_tensor(out=a2i, in0=a2i, in1=t2, op=add)
                nc.vector.tensor_tensor(
                    out=ar, in0=a2r[:, :, 0:NB], in1=a2r[:, :, NB:BANDW], op=sub
                )
                nc.vector.tensor_tensor(
                    out=ai, in0=a2i[:, :, 0:NB], in1=a2i[:, :, NB:BANDW], op=add
                )
                # interleave + store (same as unpaired path)
                obv = ob.rearrange("p (s w c) -> p s w c", w=NB, c=2)
                nc.scalar.copy(out=obv[:, :, :, 0], in_=ar)
                nc.scalar.copy(out=obv[:, :, :, 1], in_=ai)
                nc.scalar.dma_start(
                    out=AP(out_h, base * ROW,
                           [[TS * ROW, 128], [ROW, TS], [1, BANDW]]),
                    in_=ob[:, :],
                )
                continue

            cv = cs16[i].rearrange("p (s pl w) -> p s pl w", pl=2 * NO, w=NB)
            tmp = pool.tile([128, TS * NB], f16, name="tmp", bufs=2)
            tm = tmp.rearrange("p (s w) -> p s w", w=NB)

            if gp_taps:
                gacc_r = pool.tile([128, TS * NB], f16, name="gacc_r", bufs=2)
                gacc_i = pool.tile([128, TS * NB], f16, name="gacc_i", bufs=2)
                gtmp = pool.tile([128, TS * NB], f16, name="gtmp", bufs=2)
                gar = gacc_r.rearrange("p (s w) -> p s w", w=NB)
                gai = gacc_i.rearrange("p (s w) -> p s w", w=NB)
                gtm = gtmp.rearrange("p (s w) -> p s w", w=NB)

            first_v = True
            first_g = True
            for k in range(NO):
                ur = xv[:, k : k + TS, 0:NB]
                ui = xv[:, k : k + TS, NB:BANDW]
                cr = cv[:, :, 2 * k, :]
                ci = cv[:, :, 2 * k + 1, :]
                if k in gp_taps:
                    eng, a_r, a_i, t_m, first = (
                        nc.gpsimd, gar, gai, gtm, first_g)
                    first_g = False
                else:
                    eng, a_r, a_i, t_m, first = (
                        nc.vector, ar, ai, tm, first_v)
                    first_v = False
                if first:
                    eng.tensor_tensor(out=a_r, in0=ur, in1=cr, op=mult)
                    eng.tensor_tensor(out=a_i, in0=ur, in1=ci, op=mult)
                else:
                    eng.tensor_tensor(out=t_m, in0=ur, in1=cr, op=mult)
                    eng.tensor_tensor(out=a_r, in0=a_r, in1=t_m, op=add)
                    eng.tensor_tensor(out=t_m, in0=ur, in1=ci, op=mult)
                    eng.tensor_tensor(out=a_i, in0=a_i, in1=t_m, op=add)
                eng.tensor_tensor(out=t_m, in0=ui, in1=ci, op=mult)
                eng.tensor_tensor(out=a_r, in0=a_r, in1=t_m, op=sub)
                eng.tensor_tensor(out=t_m, in0=ui, in1=cr, op=mult)
                eng.tensor_tensor(out=a_i, in0=a_i, in1=t_m, op=add)
            if gp_taps:
                # merge the GpSimd partial accumulators (on DVE)
                nc.vector.tensor_tensor(out=ar, in0=ar, in1=gar, op=add)
                nc.vector.tensor_tensor(out=ai, in0=ai, in1=gai, op=add)

            # interleave (fp16 -> fp32, stride-2 dest) on the scalar engine
            obv = ob.rearrange("p (s w c) -> p s w c", w=NB, c=2)
            nc.scalar.copy(out=obv[:, :, :, 0], in_=ar)
            nc.scalar.copy(out=obv[:, :, :, 1], in_=ai)

            # store band (first 192 floats of each output row), scalar ring
            nc.scalar.dma_start(
                out=AP(out_h, base * ROW, [[TS * ROW, 128], [ROW, TS], [1, BANDW]]),
                in_=ob[:, :],
            )

        # remaining pass-through chunks after all loads
        emit_pt(npt)

    with TileContext(nc) as tc:
        with tc.tile_pool(name="pool", bufs=1) as pool:
            for _ in range(repeat):
                emit_body(nc, tc, pool)

    nc.compile()
    return nc


def _get_module(repeat: int = 1, ts_list=None, pt_after=None, gp_taps=None,
                npt=None, paired=None):
    key = (repeat, tuple(ts_list) if ts_list else None,
           tuple(pt_after) if pt_after is not None else None,
           tuple(gp_taps) if gp_taps is not None else None, npt, paired)
    if key not in _CACHE:
        _CACHE[key] = _build_module(repeat, ts_list, pt_after, gp_taps, npt,
                                    paired)
    return _CACHE[key]


def _stage_inputs(spec: np.ndarray, coef: np.ndarray):
    """Host-side staging: lossless layout reorders + zero padding only."""
    spec = np.ascontiguousarray(spec, dtype=np.float32)
    coef = np.ascontiguousarray(coef, dtype=np.float32)
    # deinterleaved band planes: [B, T, 2, 96] -> per frame xr(96) xi(96)
    specb = np.zeros((B, TP + PAD, BANDW), np.float32)
    specb[:, PAD : PAD + T] = (
        spec[:, :, :NB, :].transpose(0, 1, 3, 2).reshape(B, T, BANDW)
    )
    # pass-through columns (bins 96..480 interleaved) = float cols 192:962
    specpt = np.zeros((B, TP, PTW), np.float32)
    specpt[:, :T] = spec.reshape(B, T, ROW)[:, :, BANDW:]
    # pair-major coef planes: per frame [cr_0, ci_0, cr_1, ci_1, ..., ci_4],
    # 96 bins each, so each tap's (cr|ci) pair is one contiguous 192 slice
    coefp = np.zeros((B, TP, NCOEF), np.float32)
    coefp[:, :T] = (
        coef.reshape(B, T, NB, 2, NO).transpose(0, 1, 4, 3, 2).reshape(B, T, NCOEF)
    )
    return specb, specpt, coefp


def kernel(spec: np.ndarray, coef: np.ndarray) -> np.ndarray:
    from concourse import bass_utils

    assert spec.shape == (B, T, F, 2) and coef.shape == (B, T, NB, 2 * NO)
    specb, specpt, coefp = _stage_inputs(spec, coef)

    nc = _get_module()
    in_maps = [
        {"specb": specb[b], "specpt": specpt[b], "coef": coefp[b]}
        for b in range(B)
    ]
    res = bass_utils.run_bass_kernel_spmd(nc, in_maps, core_ids=list(range(B)))
    out = np.empty((B, T, F, 2), np.float32)
    for b in range(B):
        out[b] = res.results[b]["out"][:T].reshape(T, F, 2)
    return out
